# revision 21
# baseline (speedup 1.0000x reference)
"""LSTM decoder with attention (image captioning) — Trainium2 Bass kernel.

Sharding: data-parallel over batch (64 images -> 8 cores x 8 images).
Collective-free; host does cheap glue (embedding gather, weight
transposes, h0/c0 init, final bias add).

Device program per core (b = 8 local images):
  pre:   enc_projT[a, (b,p)] = wenc @ IF.T + (wenc_b + wdec_b)
  loop (t = 0..19, serial):
         hprojT = wdec @ h
         att = tanh(enc_projT + hprojT)      (DVE pre-add, wide Act tanh)
         e_T[q, b] = V . att  ->  softmax    (exp on Act, same table as tanh)
         ctx[e, b] = sum_p w[b,p] IF[b,p,e]  (PE, p-partitioned IF copy)
         G = W_ih_ctx.T@ctx + W_hh@h + embproj[t]   (one PSUM bank, gate-
                                                     grouped columns)
         gates: tanh(0.5x) once over i,f,o block + tanh over g block;
         cell via DVE affine_mul (sigma(x)*y = (tanh(x/2)*.5+.5)*y)
  tail:  logits = H.T @ fc_w.T  (fc weights streamed in vocab chunks)

All matmuls bf16 (fp32 accumulation); h-state and logits bf16.
"""

import os
import sys
import numpy as np

for _p in ("/opt/trn_rl_repo",):
    if _p not in sys.path and os.path.isdir(_p):
        sys.path.insert(0, _p)

import ml_dtypes  # noqa: E402

import concourse.bass as bass  # noqa: E402
import concourse.tile as tile  # noqa: E402
from concourse import bacc, mybir  # noqa: E402
from concourse.bass_utils import run_bass_kernel_spmd  # noqa: E402
from concourse.alu_op_type import AluOpType  # noqa: E402

AF = mybir.ActivationFunctionType
F32 = mybir.dt.float32
BF16 = mybir.dt.bfloat16
FP8 = mybir.dt.float8e4
BF = ml_dtypes.bfloat16
F8 = ml_dtypes.float8_e4m3
WENC_SCALE = 32.0

# problem shapes (hardcoded)
VOCAB, ENC, EMB, DEC, ATT = 10000, 2048, 512, 512, 512
B, P, S = 64, 196, 20
NCORES = 8
NB = B // NCORES          # 8 images per core
P1, P2 = 128, P - 128     # q tiles per image: 128 + 68
NE = ENC // 128           # 16
NA = ATT // 128           # 4
ND = DEC // 128           # 4
D4 = 4 * DEC              # 2048
NM = D4 // 128            # 16 gate m-tiles
BP = NB * P               # 1568 packed (b, p) columns
NVC = 20                  # vocab chunks
VC = VOCAB // NVC         # 500
CCH = BP // 4             # 392-wide enc_proj chunks

# gate PSUM bank column order: i-tiles, f-tiles, o-tiles, g-tiles
# (m = gate*ND + r with gates ordered i,f,g,o in W_ih)
QORDER = [0, 1, 2, 3, 4, 5, 6, 7, 12, 13, 14, 15, 8, 9, 10, 11]

_CACHE = {}
TRACE = False  # set by test.py to capture an NTFF profile


def _build_nc():
    if "nc" in _CACHE:
        return _CACHE["nc"]

    nc = bacc.Bacc(
        "TRN2",
        target_bir_lowering=False,
        debug=False,
        enable_asserts=False,
        num_devices=NCORES,
    )

    def din(name, shape, dt=BF16):
        return nc.dram_tensor(name, shape, dt, kind="ExternalInput").ap()

    ift_d = din("ift", [128, NE * BP], FP8)         # IF.T  [r, (e, b, p)]
    ift2_d = din("ift2", [128, 2 * NB * ENC])       # IF    [q, (b, j, e)]
    wenct_d = din("wenct", [128, NE * ATT], FP8)    # wenc.T [r, (e, a)] x32
    wct_d = din("wct", [128, NE * D4])              # Wc.T  [r, (e, d4)] q-ord
    whht_d = din("whht", [128, ND * D4])            # W_hh.T [r, (k, d4)]
    wdect_d = din("wdect", [128, ND * ATT])         # wdec.T [r, (k, a)]
    vt_d = din("vt", [128, NA])                     # V_w.T
    ept_d = din("ept", [128, S * 128])              # embprojT [r, (t, q, b)]
    i128_d = din("i128", [128, 128])                # identity bf16
    fct_d = din("fct", [128, NVC * ND * VC])        # fc_w.T [r, (c, k, v)]
    h0_d = din("h0", [128, ND * NB])                # bf16 (r, b) slab
    c0_d = din("c0", [128, 4 * NB], F32)            # (r, b) slab
    encb2_d = din("encb2", [128, NA], F32)          # wenc_b + wdec_b
    out_d = nc.dram_tensor("out", [S * NB, VOCAB], BF16,
                           kind="ExternalOutput").ap()

    with tile.TileContext(nc) as tc:
        from contextlib import ExitStack

        with ExitStack() as glob_ctx:
            gp = glob_ctx.enter_context(tc.tile_pool(name="glob", bufs=1))
            # persistent state / loop constants
            ift2T = gp.tile([128, 2 * NB * ENC], BF16, name="ift2T")
            ift2 = [ift2T[:, bj * ENC:(bj + 1) * ENC]
                    for bj in range(2 * NB)]
            whhtT = gp.tile([128, ND * D4], BF16, name="whhtT")
            whht = [whhtT[:, k * D4:(k + 1) * D4] for k in range(ND)]
            wdectT = gp.tile([128, ND * ATT], BF16, name="wdectT")
            wdect = [wdectT[:, k * ATT:(k + 1) * ATT] for k in range(ND)]
            encp = [gp.tile([128, BP], BF16, name=f"encp{i}", tag=f"encp{i}")
                    for i in range(NA)]
            att = [gp.tile([128, BP], BF16, name=f"att{i}", tag=f"att{i}")
                   for i in range(NA)]
            ept = gp.tile([128, S * 128], BF16, name="ept")
            HBall = gp.tile([128, ND * (S + 1) * NB], BF16, name="HBall")
            HROW = (S + 1) * NB
            HB = [HBall[:, r * HROW:(r + 1) * HROW] for r in range(ND)]
            HB3 = HBall.rearrange("p (r tb) -> p r tb", r=ND)
            cT = gp.tile([128, 4 * NB], F32, name="cT")
            vtT = gp.tile([128, NA], BF16, name="vtT")
            vt = [vtT[:, i:i + 1] for i in range(NA)]
            encbT = gp.tile([128, NA], F32, name="encbT")
            encb2 = [encbT[:, i:i + 1] for i in range(NA)]
            i128 = gp.tile([128, 128], BF16, name="i128")
            ones_col = gp.tile([128, 1], BF16, name="ones_col")
            et_exp = gp.tile([128, 2 * NB], BF16, name="etx")
            rsum_sb = gp.tile([1, NB], F32, name="rsum_sb")
            hp_sb = gp.tile([128, NA * NB], F32, name="hp_sb")
            rs_sb = gp.tile([128, NB], F32, name="rs_sb")
            tg = gp.tile([128, 128], F32, name="tg")
            th = gp.tile([128, 4 * NB], F32, name="th")
            Acl = gp.tile([128, 4 * NB], F32, name="Acl")
            Bcl = gp.tile([128, 4 * NB], F32, name="Bcl")
            acc0 = gp.tile([128, 1], F32, name="acc0")
            acc1 = gp.tile([128, 1], F32, name="acc1")
            acc2 = gp.tile([128, 1], F32, name="acc2")

            # first half of Wc.T lives in glob: its DMA must not wait on
            # the enc_proj input space being freed (SBUF WAR)
            wctA = gp.tile([128, (NE // 2) * D4], BF16, name="wctA")

            # ---------------- pre-loop: enc_proj ----------------
            # DMA priority: ift/wenct feed enc_proj immediately; then the
            # small recurrence inputs; ift2/wct follow (needed ~t0+10us).
            with tc.tile_pool(name="pre", bufs=1) as pre, \
                 tc.tile_pool(name="pspre", bufs=4, space="PSUM") as pspre:
                iftT = pre.tile([128, NE * BP], FP8, name="iftT")
                ift = [iftT[:, k * BP:(k + 1) * BP] for k in range(NE)]
                wenctT = pre.tile([128, NE * ATT], FP8, name="wenctT")
                wenct = [wenctT[:, k * ATT:(k + 1) * ATT] for k in range(NE)]
                nc.sync.dma_start(out=wenctT, in_=wenct_d)
                nc.sync.dma_start(out=iftT, in_=ift_d)
                nc.sync.dma_start(out=i128, in_=i128_d)
                nc.sync.dma_start(out=ept, in_=ept_d)
                nc.sync.dma_start(out=whhtT, in_=whht_d)
                nc.sync.dma_start(out=wdectT, in_=wdect_d)
                nc.sync.dma_start(out=HB3[:, :, 0:NB],
                                  in_=h0_d.rearrange("p (r b) -> p r b", r=ND))
                nc.sync.dma_start(out=cT, in_=c0_d)
                nc.sync.dma_start(out=vtT, in_=vt_d)
                nc.sync.dma_start(out=encbT, in_=encb2_d)
                nc.sync.dma_start(out=ift2T, in_=ift2_d)
                nc.sync.dma_start(out=wctA,
                                  in_=wct_d[:, 0:(NE // 2) * D4])
                nc.vector.memset(ones_col, 1.0)
                for i in range(NA):
                    for c in range(4):
                        ps = pspre.tile([128, CCH], F32, name="eps", tag="mm")
                        for k in range(NE):
                            nc.tensor.matmul(
                                ps, wenct[k][:, i * 128:(i + 1) * 128],
                                ift[k][:, c * CCH:(c + 1) * CCH],
                                start=(k == 0), stop=(k == NE - 1))
                        nc.vector.tensor_scalar(
                            encp[i][:, c * CCH:(c + 1) * CCH], ps,
                            1.0 / WENC_SCALE, encb2[i],
                            mybir.AluOpType.mult, mybir.AluOpType.add)

            # ---------------- recurrence ----------------
            with tc.tile_pool(name="wpool", bufs=1) as wp, \
                 tc.tile_pool(name="psG", bufs=2, space="PSUM") as psG, \
                 tc.tile_pool(name="psC", bufs=2, space="PSUM") as psC, \
                 tc.tile_pool(name="psS", bufs=2, space="PSUM") as psS, \
                 tc.tile_pool(name="ai", bufs=2) as aip, \
                 tc.tile_pool(name="cw", bufs=2) as cw, \
                 tc.tile_pool(name="fc", bufs=1) as fcp, \
                 tc.tile_pool(name="pst", bufs=2, space="PSUM") as pst:
                wctB = wp.tile([128, (NE // 2) * D4], BF16, name="wctB")
                wct = [wctA[:, k * D4:(k + 1) * D4] for k in range(NE // 2)] \
                    + [wctB[:, k * D4:(k + 1) * D4] for k in range(NE // 2)]
                nc.sync.dma_start(out=wctB,
                                  in_=wct_d[:, (NE // 2) * D4:NE * D4])
                # fc-chunk staging: in the wpool (its DMAs tolerate the
                # SBUF WAR wait; first needed at the tail)
                fch_t = [wp.tile([128, ND * VC], BF16, name=f"fch{s_}",
                                 tag=f"fch{s_}") for s_ in range(3)]
                # prefetch the first fc chunks: DMA engines idle during the
                # recurrence, and the tail's loads sit behind the per-step
                # pool ops in the in-order gpsimd queue
                for c in range(3):
                    nc.sync.dma_start(
                        out=fch_t[c],
                        in_=fct_d[:, c * ND * VC:(c + 1) * ND * VC])

                # the wide exp reads etp[1] rows past P2 (never matmul-
                # written); zero them once in both SM ring buffers so
                # exp(stale) can't produce NaN/Inf on hardware
                for _ in range(2):
                    SM0 = psS.tile([128, 512], F32, name="SM", tag="SM")
                    nc.vector.memset(SM0[:, 40:48], 0.0)

                for t in range(S):
                    hcol = t * NB

                    def h(r):
                        return HB[r][:, hcol:hcol + NB]  # noqa: B023

                    # small per-step PSUM regions packed into one bank
                    SM = psS.tile([128, 512], F32, name="SM", tag="SM")
                    hp = SM[:, 0:NA * NB]
                    etp = [SM[:, 32 + 8 * j:32 + 8 * (j + 1)] for j in range(2)]
                    sum_ps = SM[0:1, 48:48 + NB]
                    rs_ps = SM[:, 56:56 + NB]

                    # PE: hproj (critical) then G init (ept + whh, off-path)
                    for i in range(NA):
                        for k in range(ND):
                            nc.tensor.matmul(
                                hp[:, i * NB:(i + 1) * NB],
                                wdect[k][:, i * 128:(i + 1) * 128], h(k),
                                start=(k == 0), stop=(k == ND - 1),
                                skip_group_check=True)
                    G = psG.tile([128, 128], F32, name="G", tag="G")
                    nc.tensor.matmul(G, i128, ept[:, t * 128:(t + 1) * 128],
                                     start=True, stop=False,
                                     skip_group_check=True)
                    for q in range(NM):
                        for k in range(ND):
                            nc.tensor.matmul(
                                G[:, q * NB:(q + 1) * NB],
                                whht[k][:, q * 128:(q + 1) * 128], h(k),
                                start=False, stop=False,
                                skip_group_check=True)

                    # DVE per-(i,b) pre-adds; per-i hp copy keeps the
                    # SBUF-scalar fast path and a short tanh-0 lead-in
                    for i in range(NA):
                        io = i * NB
                        nc.vector.tensor_copy(
                            out=hp_sb[:, io:io + NB], in_=hp[:, io:io + NB])
                        ain = aip.tile([128, BP], BF16, name="ain", tag="ain")
                        for b in range(NB):
                            lo = b * P
                            nc.vector.tensor_scalar_add(
                                ain[:, lo:lo + P], encp[i][:, lo:lo + P],
                                hp_sb[:, io + b:io + b + 1])
                        nc.scalar.activation(att[i], ain, AF.Tanh)
                    # e-dots: b outer so each PSUM column's start..stop run is
                    # contiguous (start=True marks the whole 2KB zero-region)
                    for b in range(NB):
                        lo = b * P
                        for i in range(NA):
                            nc.tensor.matmul(
                                etp[0][:, b:b + 1], att[i][:, lo:lo + P1],
                                vt[i], start=(i == 0), stop=(i == NA - 1),
                                skip_group_check=True)
                        for i in range(NA):
                            nc.tensor.matmul(
                                etp[1][0:P2, b:b + 1],
                                att[i][:, lo + P1:lo + P], vt[i],
                                start=(i == 0), stop=(i == NA - 1),
                                skip_group_check=True)

                    # softmax (no max-subtract: |e| <= sum|V| ~ 11)
                    nc.scalar.activation(et_exp, SM[:, 32:48], AF.Exp)
                    nc.tensor.matmul(sum_ps, ones_col, et_exp[:, 0:NB],
                                     start=True, stop=False,
                                     skip_group_check=True)
                    nc.tensor.matmul(sum_ps, ones_col[0:P2],
                                     et_exp[0:P2, NB:2 * NB],
                                     start=False, stop=True,
                                     skip_group_check=True)
                    nc.vector.reciprocal(rsum_sb, sum_ps)
                    nc.gpsimd.partition_broadcast(rs_sb, rsum_sb)

                    # PE: unnormalized ctx'[e, b] = sum_p exp_e[b,p] IF[b,p,e]
                    # (softmax 1/s folded into the PSUM->SBUF copy below)
                    CT = psC.tile([128, 128], F32, name="CT", tag="CT")
                    for ec in range(NE):
                        es = ec * 128
                        for b in range(NB):
                            col = ec * NB + b
                            nc.tensor.matmul(
                                CT[:, col:col + 1],
                                ift2[2 * b][:, es:es + 128],
                                et_exp[:, b:b + 1], start=True, stop=False,
                                skip_group_check=True)
                            nc.tensor.matmul(
                                CT[:, col:col + 1],
                                ift2[2 * b + 1][:, es:es + 128],
                                et_exp[:, NB + b:NB + b + 1],
                                start=False, stop=True,
                                skip_group_check=True)
                    ctx_sb = cw.tile([128, 128], BF16, name="ctx", tag="ctx")
                    for cc in range(4):
                        sl = slice(cc * 32, (cc + 1) * 32)
                        nc.vector.tensor_tensor(
                            out=ctx_sb[:, sl].rearrange(
                                "p (e b) -> p e b", e=4),
                            in0=CT[:, sl].rearrange("p (e b) -> p e b", e=4),
                            in1=rs_sb.unsqueeze(1).broadcast_to((128, 4, NB)),
                            op=mybir.AluOpType.mult)

                    # PE: gates += Wc.T @ ctx (ec outer: overlap with the
                    # chunked normalize above)
                    for ec in range(NE):
                        for q in range(NM):
                            nc.tensor.matmul(
                                G[:, q * NB:(q + 1) * NB],
                                wct[ec][:, q * 128:(q + 1) * 128],
                                ctx_sb[:, ec * NB:(ec + 1) * NB],
                                start=False, stop=(ec == NE - 1),
                                skip_group_check=True)

                    # gate activations: tanh for g first (unblocks Bcl
                    # as soon as the i,f,o tanh(x/2) lands), then i,f,o
                    nc.scalar.activation(tg[:, 96:128], G[:, 96:128], AF.Tanh)
                    nc.scalar.activation(tg[:, 0:96], G[:, 0:96], AF.Tanh,
                                         scale=0.5)
                    # cell: sigma(x)*y = (tanh(x/2)*0.5+0.5)*y
                    nc.vector.affine_mul_reduce(Acl, acc0, tg[:, 32:64], cT,
                                                0.5, 0.5)
                    nc.vector.affine_mul_reduce(Bcl, acc1, tg[:, 0:32],
                                                tg[:, 96:128], 0.5, 0.5)
                    nc.vector.tensor_add(cT, Acl, Bcl)
                    nc.scalar.activation(th, cT, AF.Tanh)
                    nc.vector.affine_mul_reduce(
                        HB3[:, :, hcol + NB:hcol + 2 * NB], acc2,
                        tg[:, 64:96].rearrange("p (r b) -> p r b", r=ND),
                        th.rearrange("p (r b) -> p r b", r=ND), 0.5, 0.5)

                # ---------------- tail: logits ----------------
                for c in range(NVC):
                    fch = fch_t[c % 3]
                    if c >= 3:
                        # gpsimd queue, one contiguous transfer per chunk
                        nc.gpsimd.dma_start(
                            out=fch,
                            in_=fct_d[:, c * ND * VC:(c + 1) * ND * VC])
                    for m0, msz in ((0, 128), (128, S * NB - 128)):
                        ps = pst.tile([128, VC], F32, name="lps", tag="l")
                        for k in range(ND):
                            nc.tensor.matmul(
                                ps[:msz], HB[k][:, NB + m0:NB + m0 + msz],
                                fch[:, k * VC:(k + 1) * VC],
                                start=(k == 0), stop=(k == ND - 1))
                        lg = fcp.tile([128, VC], BF16, name="lg", tag="lg",
                                      bufs=4)
                        if c % 2 == 0:
                            nc.scalar.copy(out=lg[:msz], in_=ps[:msz])
                        else:
                            nc.vector.tensor_copy(out=lg[:msz], in_=ps[:msz])
                        nc.sync.dma_start(
                            out=out_d[m0:m0 + msz, c * VC:(c + 1) * VC],
                            in_=lg[:msz])

    nc.compile()
    _CACHE["nc"] = nc
    return nc


def _host_weights(wenc_w, wenc_b, wdec_w, wdec_b, V_w, W_ih, W_hh, fc_w):
    cols = np.concatenate(
        [np.arange(m * 128, (m + 1) * 128) for m in QORDER])
    Wc = np.asarray(W_ih, np.float32)[:, :ENC]
    wct = np.ascontiguousarray(
        Wc.T[:, cols].astype(BF).reshape(NE, 128, D4)
        .transpose(1, 0, 2).reshape(128, NE * D4))
    whht = np.ascontiguousarray(
        np.asarray(W_hh, np.float32).T[:, cols].astype(BF)
        .reshape(ND, 128, D4).transpose(1, 0, 2).reshape(128, ND * D4))
    wenct = np.ascontiguousarray(
        (np.asarray(wenc_w, np.float32).T * WENC_SCALE).astype(F8)
        .reshape(NE, 128, ATT).transpose(1, 0, 2).reshape(128, NE * ATT))
    wdect = np.ascontiguousarray(
        np.asarray(wdec_w, np.float32).T.astype(BF)
        .reshape(ND, 128, ATT).transpose(1, 0, 2).reshape(128, ND * ATT))
    vtt = np.ascontiguousarray(
        np.asarray(V_w, np.float32)[0].astype(BF).reshape(NA, 128)
        .T.copy())
    fct = np.ascontiguousarray(
        np.asarray(fc_w, np.float32).T).astype(BF).reshape(ND, 128, NVC, VC)
    fct = np.ascontiguousarray(
        fct.transpose(1, 2, 0, 3).reshape(128, NVC * ND * VC))
    encb2 = np.ascontiguousarray(
        (np.asarray(wenc_b, np.float32)
         + np.asarray(wdec_b, np.float32)).reshape(NA, 128).T)
    i128 = np.eye(128, dtype=BF)
    return dict(wct=wct, whht=whht, wenct=wenct, wdect=wdect, vt=vtt,
                fct=fct, encb2=encb2, i128=i128)


def _prep_core_inputs(image_feat, embproj, h0, c0, wargs, core):
    bs = slice(core * NB, (core + 1) * NB)
    imf = image_feat[bs]                                # [8, 196, 2048]
    ift = np.ascontiguousarray(
        imf.transpose(2, 0, 1).reshape(ENC, BP).astype(F8)
        .reshape(NE, 128, BP).transpose(1, 0, 2).reshape(128, NE * BP))
    ift2 = np.zeros((2 * NB, 128, ENC), BF)
    for b in range(NB):
        ift2[2 * b] = imf[b, 0:128, :].astype(BF)
        ift2[2 * b + 1][0:P2] = imf[b, 128:P, :].astype(BF)
    ift2 = np.ascontiguousarray(
        ift2.transpose(1, 0, 2).reshape(128, 2 * NB * ENC))
    # embproj [8, 20, 2048] -> [r, (t, q, b)]
    ep = embproj[bs].transpose(2, 1, 0).reshape(NM, 128, S, NB)
    ep = ep[QORDER]                                     # [q, r, t, b]
    ept = np.ascontiguousarray(
        ep.transpose(1, 2, 0, 3).reshape(128, S * 128)).astype(BF)
    h0t = np.ascontiguousarray(
        h0[bs].T.astype(BF).reshape(ND, 128, NB).transpose(1, 0, 2)
        .reshape(128, ND * NB))
    c0t = np.ascontiguousarray(
        c0[bs].T.reshape(ND, 128, NB).transpose(1, 0, 2).reshape(
            128, ND * NB)).astype(np.float32)
    return dict(ift=ift, ift2=ift2, ept=ept, h0=h0t, c0=c0t, **wargs)


def kernel(image_feat, captions_ids, wenc_w, wenc_b, wdec_w, wdec_b,
           V_w, V_b, embed_w, h0_w, h0_b, c0_w, c0_b,
           W_ih, b_ih, W_hh, b_hh, fc_w, fc_b):
    image_feat = np.asarray(image_feat, np.float32)
    ids = np.asarray(captions_ids).astype(np.int64)

    # host-side glue (cheap, not on the device critical path)
    emb_seq = np.asarray(embed_w, np.float32)[ids]            # [B, S, EMB]
    We = np.asarray(W_ih, np.float32)[:, ENC:]                # [D4, EMB]
    embproj = emb_seq @ We.T + (np.asarray(b_ih) + np.asarray(b_hh))
    avg = image_feat.mean(axis=1)
    h0 = np.maximum(avg @ np.asarray(h0_w, np.float32).T + h0_b, 0.0)
    c0 = np.maximum(avg @ np.asarray(c0_w, np.float32).T + c0_b, 0.0)

    wargs = _host_weights(wenc_w, wenc_b, wdec_w, wdec_b, V_w, W_ih,
                          W_hh, fc_w)

    nc = _build_nc()
    in_maps = [
        _prep_core_inputs(image_feat, embproj, h0, c0, wargs, c)
        for c in range(NCORES)
    ]
    res = run_bass_kernel_spmd(nc, in_maps, core_ids=list(range(NCORES)),
                               trace=TRACE)
    _CACHE["last_results"] = res

    preds = np.empty((B, S, VOCAB), np.float32)
    for c in range(NCORES):
        lg = res.results[c]["out"].astype(np.float32).reshape(S, NB, VOCAB)
        preds[c * NB:(c + 1) * NB] = lg.transpose(1, 0, 2)
    preds += np.asarray(fc_b, np.float32)
    return preds


if __name__ == "__main__":
    sys.path.insert(0, os.path.dirname(os.path.abspath(__file__)))
    import reference

    inputs = reference.setup_inputs()
    inputs = {k: np.asarray(v) for k, v in inputs.items()}
    expected = np.asarray(reference.reference(**inputs))
    actual = kernel(**inputs)
    err = np.abs(actual - expected)
    rel = np.linalg.norm(actual - expected) / np.linalg.norm(expected)
    print("max abs err:", err.max(), "rel:", rel)
